# revision 11
# baseline (speedup 1.0000x reference)
"""AFT-Full (nn_AFT_Full) Trainium2 Bass kernel, 8-core SPMD, batch-sharded.

Math note: in the reference, w_bias has shape [1,T,T] and max over dim 0 is the
identity, so exp_wb == exp(0) == 1 and the [T,T] matmuls reduce to column sums
over T (u/vp are unused):
    num[b,h] = sum_t exp(k[b,t,h] - m[t,h]) * v[b,t,h]
    den[b,h] = sum_t exp(k[b,t,h] - m[t,h])
    out = (sigmoid(q) * num/den) @ Wo + bo
where m = max over the FULL batch of k -> cross-core AllReduce(max).

We compute E0 = exp(k + bk) directly (fused into the PSUM->SBUF copy), take
M = max_b E0 (exp is monotone, so this is exp(m)), AllReduce(max) on M, and use
s = 1/M so that exp(k - m) == E0 * s.
"""
import os
import sys

sys.path.insert(0, "/opt/trn_rl_repo")

import numpy as np

# ---- problem constants (hardcoded per spec) ----
B, Hh, Ww, C = 64, 24, 24, 768
HID = 576
T = Hh * Ww          # 576
N_CORES = 8
B_LOC = B // N_CORES  # 8
R = B_LOC * T         # 4608 rows per core
WIN = 512             # row window
NWIN = R // WIN       # 9
NRC = WIN // 128      # 4 row chunks per window
NCC = C // 128        # 6 contraction chunks for projections
HC_SIZES = [128, 128, 128, 128, 64]   # HID = 576 partition chunks
NOUT_HALF = 384       # out matmul free-dim split (768 = 2*384)

_CACHE = {}
LAST_EXEC_NS = None


def _build(stage="full"):
    import concourse.bass as bass
    import concourse.mybir as mybir
    from concourse import bacc, tile

    def stage_lt(s):
        order = ["p1w1", "p1", "mx", "cc", "nd", "p2w1", "full"]
        return order.index(stage) < order.index(s)

    f32 = mybir.dt.float32
    bf16 = mybir.dt.bfloat16
    AF = mybir.ActivationFunctionType

    nc = bacc.Bacc("TRN2", target_bir_lowering=False, debug=False,
                   num_devices=N_CORES)

    x = nc.dram_tensor("x", [R, C], f32, kind="ExternalInput").ap()
    Wq = nc.dram_tensor("Wq", [C, HID], f32, kind="ExternalInput").ap()
    Wk = nc.dram_tensor("Wk", [C, HID], f32, kind="ExternalInput").ap()
    Wv = nc.dram_tensor("Wv", [C, HID], f32, kind="ExternalInput").ap()
    bq = nc.dram_tensor("bq", [HID], f32, kind="ExternalInput").ap()
    bk = nc.dram_tensor("bk", [HID], f32, kind="ExternalInput").ap()
    bv = nc.dram_tensor("bv", [HID], f32, kind="ExternalInput").ap()
    Wo = nc.dram_tensor("Wo", [HID, C], f32, kind="ExternalInput").ap()
    bo = nc.dram_tensor("bo", [C], f32, kind="ExternalInput").ap()
    ident = nc.dram_tensor("ident", [128, 128], f32, kind="ExternalInput").ap()
    out = nc.dram_tensor("out", [R, C], f32, kind="ExternalOutput").ap()

    with tile.TileContext(nc) as tc:
        with (
            tc.tile_pool(name="const", bufs=1) as cpool,
            tc.tile_pool(name="resident", bufs=1) as rpool,
            tc.tile_pool(name="xn", bufs=2) as xnpool,
            tc.tile_pool(name="xt", bufs=2) as xtpool,
            tc.tile_pool(name="qy", bufs=2) as qypool,
            tc.tile_pool(name="ob", bufs=2) as obpool,
            tc.tile_pool(name="pt", bufs=2, space="PSUM") as ptpool,
            tc.tile_pool(name="pm", bufs=3, space="PSUM") as pmpool,
            tc.tile_pool(name="po", bufs=3, space="PSUM") as popool,
            tc.tile_pool(name="dram", bufs=1, space="DRAM") as dpool,
        ):
            # ---------- constants ----------
            ident_sb = cpool.tile([128, 128], bf16, tag="ident", name="ident")
            nc.gpsimd.dma_start(ident_sb[:], ident[:])

            def load_w(name, w_ap):
                tiles = []
                for cc in range(NCC):
                    t = cpool.tile([128, HID], bf16, tag=f"{name}_{cc}", name=f"{name}_{cc}")
                    nc.gpsimd.dma_start(t[:], w_ap[cc * 128:(cc + 1) * 128, :])
                    tiles.append(t)
                return tiles

            Wq_sb = load_w("Wq", Wq)
            Wk_sb = load_w("Wk", Wk)
            Wv_sb = load_w("Wv", Wv)

            # Wo extended with bo as an extra contraction row (ones trick)
            Wo_sb = []
            for kc, ksz in enumerate(HC_SIZES):
                psz = ksz + 1 if kc == 4 else ksz
                t = cpool.tile([psz, C], bf16, tag=f"Wo_{kc}", name=f"Wo_{kc}")
                nc.gpsimd.dma_start(t[0:ksz, :], Wo[kc * 128:kc * 128 + ksz, :])
                if kc == 4:
                    nc.gpsimd.dma_start(t[ksz:ksz + 1, :], bo[None, :])
                Wo_sb.append(t)

            def load_bias(name, b_ap):
                tiles = []
                for hc, hsz in enumerate(HC_SIZES):
                    t = cpool.tile([hsz, 1], f32, tag=f"{name}_{hc}", name=f"{name}_{hc}")
                    nc.sync.dma_start(t[:], b_ap[hc * 128:hc * 128 + hsz][:, None])
                    tiles.append(t)
                return tiles

            bq_sb = load_bias("bq", bq)
            bk_sb = load_bias("bk", bk)
            bv_sb = load_bias("bv", bv)

            # ---------- resident tensors ----------
            E0 = [rpool.tile([hsz, R], bf16, tag=f"E0_{hc}", name=f"E0_{hc}")
                  for hc, hsz in enumerate(HC_SIZES)]
            Vs = [rpool.tile([hsz, R], bf16, tag=f"V_{hc}", name=f"V_{hc}")
                  for hc, hsz in enumerate(HC_SIZES)]
            Mx = [rpool.tile([hsz, T], f32, tag=f"M_{hc}", name=f"M_{hc}")
                  for hc, hsz in enumerate(HC_SIZES)]
            Sx = [rpool.tile([hsz, T], f32, tag=f"S_{hc}", name=f"S_{hc}")
                  for hc, hsz in enumerate(HC_SIZES)]
            den = [rpool.tile([hsz, B_LOC], f32, tag=f"den_{hc}", name=f"den_{hc}")
                   for hc, hsz in enumerate(HC_SIZES)]
            num = [rpool.tile([hsz, B_LOC], f32, tag=f"num_{hc}", name=f"num_{hc}")
                   for hc, hsz in enumerate(HC_SIZES)]
            rr = [rpool.tile([hsz, B_LOC], f32, tag=f"r_{hc}", name=f"r_{hc}")
                  for hc, hsz in enumerate(HC_SIZES)]

            def load_window_xt(w):
                """DMA-cast x rows [w*WIN, (w+1)*WIN) and transpose to
                xt[c_part, cc*WIN + r] layout."""
                xn = xnpool.tile([128, NRC * C], bf16, tag="xn", name="xn")
                src = x[w * WIN:(w + 1) * WIN, :].rearrange(
                    "(n p) c -> p n c", p=128)
                nc.gpsimd.dma_start(
                    xn[:].rearrange("p (n c) -> p n c", c=C), src)
                xt = xtpool.tile([128, NCC * WIN], bf16, tag="xt", name="xt")
                for rc in range(NRC):
                    for cc in range(NCC):
                        pt = ptpool.tile([128, 128], bf16, tag="pt", name="pt")
                        nc.tensor.transpose(
                            pt[:],
                            xn[:, rc * C + cc * 128: rc * C + (cc + 1) * 128],
                            ident_sb[:])
                        nc.vector.tensor_copy(
                            xt[:, cc * WIN + rc * 128: cc * WIN + (rc + 1) * 128],
                            pt[:])
                return xt

            def project(xt, w_tiles, hc, hsz):
                pm = pmpool.tile([hsz, WIN], mybir.dt.float32, tag="pm", name="pm")
                for cc in range(NCC):
                    nc.tensor.matmul(
                        pm[:],
                        w_tiles[cc][:, hc * 128: hc * 128 + hsz],
                        xt[:, cc * WIN:(cc + 1) * WIN],
                        start=(cc == 0), stop=(cc == NCC - 1))
                return pm

            def dummy_out():
                d = obpool.tile([128, C], mybir.dt.float32, tag="ob", name="ob")
                nc.vector.memset(d[:], 0.0)
                nc.sync.dma_start(out[0:128, :], d[:])

            # ---------- pass 1: k (as exp) and v ----------
            for w in range(1 if stage == "p1w1" else NWIN):
                xt = load_window_xt(w)
                for hc, hsz in enumerate(HC_SIZES):
                    pm = project(xt, Wk_sb, hc, hsz)
                    nc.scalar.activation(
                        E0[hc][:, w * WIN:(w + 1) * WIN], pm[:],
                        AF.Exp, bias=bk_sb[hc][:])
                for hc, hsz in enumerate(HC_SIZES):
                    pm = project(xt, Wv_sb, hc, hsz)
                    nc.scalar.activation(
                        Vs[hc][:, w * WIN:(w + 1) * WIN], pm[:],
                        AF.Identity, bias=bv_sb[hc][:])

            # ---------- local batch max (tree over 8 batches) ----------
            for hc, hsz in (enumerate(HC_SIZES) if not stage_lt("mx") else []):
                mt = rpool.tile([128, 6 * T], bf16, tag="mt", name="mt")  # tree scratch
                ev = E0[hc][:].rearrange("p (b t) -> p b t", t=T)
                # level 1: 8 -> 4
                nc.vector.tensor_max(
                    mt[0:hsz, 0:4 * T].rearrange("p (b t) -> p b t", t=T),
                    ev[:, 0:4, :], ev[:, 4:8, :])
                # level 2: 4 -> 2
                nc.vector.tensor_max(
                    mt[0:hsz, 4 * T:6 * T].rearrange("p (b t) -> p b t", t=T),
                    mt[0:hsz, 0:2 * T].rearrange("p (b t) -> p b t", t=T),
                    mt[0:hsz, 2 * T:4 * T].rearrange("p (b t) -> p b t", t=T))
                # level 3: 2 -> 1
                nc.vector.tensor_max(
                    Mx[hc][:, :], mt[0:hsz, 4 * T:5 * T], mt[0:hsz, 5 * T:6 * T])

            # ---------- AllReduce(max) over batch dim ----------
            if stage_lt("cc"):
                dummy_out()
                stage_done = True
            else:
                stage_done = False
            if not stage_done:
                bounce_in = dpool.tile([HID, T], f32, name="bounce_in")
                bounce_out = dpool.tile([HID, T], f32, name="bounce_out",
                                        addr_space="Shared")
                for hc, hsz in enumerate(HC_SIZES):
                    nc.sync.dma_start(
                        bounce_in[hc * 128:hc * 128 + hsz, :], Mx[hc][:])
                nc.gpsimd.collective_compute(
                    "AllReduce",
                    mybir.AluOpType.max,
                    replica_groups=[list(range(N_CORES))],
                    ins=[bounce_in.opt()],
                    outs=[bounce_out.opt()],
                )
                for hc, hsz in enumerate(HC_SIZES):
                    nc.sync.dma_start(
                        Mx[hc][:], bounce_out[hc * 128:hc * 128 + hsz, :])
                    nc.vector.reciprocal(Sx[hc][:], Mx[hc][:])

            # ---------- num/den (fused multiply+reduce per batch) ----------
            if stage_lt("nd") and not stage_done:
                dummy_out()
                stage_done = True
            for hc, hsz in (enumerate(HC_SIZES) if not stage_done else []):
                ev = E0[hc][:].rearrange("p (b t) -> p b t", t=T)
                # E = E0 * s  (per-batch slices; s is shared over b)
                for b in range(B_LOC):
                    e_b = E0[hc][:, b * T:(b + 1) * T]
                    nc.vector.tensor_mul(e_b, e_b, Sx[hc][:])
                nc.vector.reduce_sum(den[hc][:], ev, axis=mybir.AxisListType.X)
                # p = E * v, num = sum_t p
                nc.vector.tensor_mul(E0[hc][:], E0[hc][:], Vs[hc][:])
                nc.vector.reduce_sum(num[hc][:], ev, axis=mybir.AxisListType.X)
                # r = num / den
                nc.vector.reciprocal(rr[hc][:], den[hc][:])
                nc.vector.tensor_mul(rr[hc][:], rr[hc][:], num[hc][:])

            # ---------- pass 2: q -> sigmoid -> y -> out ----------
            if stage_lt("p2w1") and not stage_done:
                dummy_out()
                stage_done = True
            for w in (range(1 if stage == "p2w1" else NWIN)
                      if not stage_done else []):
                xt = load_window_xt(w)
                qy = []
                for hc, hsz in enumerate(HC_SIZES):
                    psz = hsz + 1 if hc == 4 else hsz
                    t = qypool.tile([psz, WIN], bf16, tag=f"qy_{hc}", name=f"qy_{hc}")
                    pm = project(xt, Wq_sb, hc, hsz)
                    nc.scalar.activation(
                        t[0:hsz, :], pm[:], AF.Sigmoid, bias=bq_sb[hc][:])
                    if hc == 4:
                        nc.vector.memset(t[hsz:hsz + 1, :], 1.0)
                    qy.append(t)
                # y = sigmoid(q) * r  (r constant over t within one batch)
                lo = w * WIN
                hi = lo + WIN
                for b in range(B_LOC):
                    s = max(lo, b * T)
                    e = min(hi, (b + 1) * T)
                    if s >= e:
                        continue
                    for hc, hsz in enumerate(HC_SIZES):
                        nc.vector.tensor_scalar_mul(
                            qy[hc][0:hsz, s - lo:e - lo],
                            qy[hc][0:hsz, s - lo:e - lo],
                            rr[hc][:, b:b + 1])
                # out = y_ext @ Wo_ext
                for rc in range(NRC):
                    ob = obpool.tile([128, C], mybir.dt.float32, tag="ob", name="ob")
                    for nn in range(2):
                        po = popool.tile([128, NOUT_HALF], mybir.dt.float32,
                                         tag="po", name="po")
                        for kc, ksz in enumerate(HC_SIZES):
                            psz = ksz + 1 if kc == 4 else ksz
                            nc.tensor.matmul(
                                po[:],
                                qy[kc][0:psz, rc * 128:(rc + 1) * 128],
                                Wo_sb[kc][0:psz,
                                          nn * NOUT_HALF:(nn + 1) * NOUT_HALF],
                                start=(kc == 0), stop=(kc == 4))
                        nc.scalar.copy(
                            ob[:, nn * NOUT_HALF:(nn + 1) * NOUT_HALF], po[:])
                    nc.sync.dma_start(
                        out[w * WIN + rc * 128: w * WIN + (rc + 1) * 128, :],
                        ob[:])

    nc.compile()
    return nc


def kernel(**inputs):
    global LAST_EXEC_NS
    from concourse import bass_utils

    if "nc" not in _CACHE:
        _CACHE["nc"] = _build()
    nc = _CACHE["nc"]

    x = np.asarray(inputs["x"], dtype=np.float32).reshape(B, T, C)
    eye = np.eye(128, dtype=np.float32)
    common = {
        "Wq": np.asarray(inputs["Wq"], np.float32),
        "Wk": np.asarray(inputs["Wk"], np.float32),
        "Wv": np.asarray(inputs["Wv"], np.float32),
        "bq": np.asarray(inputs["bq"], np.float32),
        "bk": np.asarray(inputs["bk"], np.float32),
        "bv": np.asarray(inputs["bv"], np.float32),
        "Wo": np.asarray(inputs["Wo"], np.float32),
        "bo": np.asarray(inputs["bo"], np.float32),
        "ident": eye,
    }
    in_maps = []
    for i in range(N_CORES):
        m = dict(common)
        m["x"] = np.ascontiguousarray(
            x[i * B_LOC:(i + 1) * B_LOC].reshape(R, C))
        in_maps.append(m)

    trace = bool(os.environ.get("KERNEL_TRACE"))
    res = bass_utils.run_bass_kernel_spmd(
        nc, in_maps, core_ids=list(range(N_CORES)), trace=trace)
    LAST_EXEC_NS = res.exec_time_ns

    shards = [res.results[i]["out"].reshape(B_LOC, Hh, Ww, C)
              for i in range(N_CORES)]
    return np.concatenate(shards, axis=0)


# revision 14
# speedup vs baseline: 1.5061x; 1.5061x over previous
"""AFT-Full (nn_AFT_Full) Trainium2 Bass kernel, 8-core SPMD, batch-sharded.

Math note: in the reference, w_bias has shape [1,T,T] and max over dim 0 is the
identity, so exp_wb == exp(0) == 1 and the [T,T] matmuls reduce to column sums
over T (u/vp are unused):
    num[b,h] = sum_t exp(k[b,t,h] - m[t,h]) * v[b,t,h]
    den[b,h] = sum_t exp(k[b,t,h] - m[t,h])
    out = (sigmoid(q) * num/den) @ Wo + bo
where m = max over the FULL batch of k -> cross-core AllReduce(max).

We compute E0 = exp(k + bk) directly (fused into the PSUM->SBUF copy), take
M = max_b E0 (exp is monotone, so this is exp(m)), AllReduce(max) on M, and use
s = 1/M so that exp(k - m) == E0 * s.

Schedule: two passes over x (k,v then q) with on-chip TE transposes; the
batch-max M is accumulated per window during pass 1 so the collective fires
immediately after; pass-2 x loads are issued before the collective trigger
(gpsimd is in-order) and num/den chunks interleave with pass-2 windows.
"""
import os
import sys

sys.path.insert(0, "/opt/trn_rl_repo")

import numpy as np

# ---- problem constants (hardcoded per spec) ----
B, Hh, Ww, C = 64, 24, 24, 768
HID = 576
T = Hh * Ww          # 576
N_CORES = 8
B_LOC = B // N_CORES  # 8
R = B_LOC * T         # 4608 rows per core
WIN = 512             # row window
NWIN = R // WIN       # 9
NRC = WIN // 128      # 4 row chunks per window
NCC = C // 128        # 6 contraction chunks for projections
HC_SIZES = [128, 128, 128, 128, 64]   # HID = 576 partition chunks
NOUT_HALF = 384       # out matmul free-dim split (768 = 2*384)

_CACHE = {}
LAST_EXEC_NS = None


def _window_segments(w):
    """Batch segments [(b, lo, hi)] of window w, window-local coords."""
    lo, hi = w * WIN, (w + 1) * WIN
    segs = []
    for b in range(B_LOC):
        s = max(lo, b * T)
        e = min(hi, (b + 1) * T)
        if s < e:
            segs.append((b, s - lo, e - lo))
    return segs


def _build():
    import concourse.bass as bass
    import concourse.mybir as mybir
    from concourse import bacc, tile

    f32 = mybir.dt.float32
    bf16 = mybir.dt.bfloat16
    AF = mybir.ActivationFunctionType

    nc = bacc.Bacc("TRN2", target_bir_lowering=False, debug=False,
                   num_devices=N_CORES)

    x = nc.dram_tensor("x", [R, C], f32, kind="ExternalInput").ap()
    Wq = nc.dram_tensor("Wq", [C, HID], f32, kind="ExternalInput").ap()
    Wk = nc.dram_tensor("Wk", [C, HID], f32, kind="ExternalInput").ap()
    Wv = nc.dram_tensor("Wv", [C, HID], f32, kind="ExternalInput").ap()
    bq = nc.dram_tensor("bq", [HID], f32, kind="ExternalInput").ap()
    bk = nc.dram_tensor("bk", [HID], f32, kind="ExternalInput").ap()
    bv = nc.dram_tensor("bv", [HID], f32, kind="ExternalInput").ap()
    Wo = nc.dram_tensor("Wo", [HID, C], f32, kind="ExternalInput").ap()
    bo = nc.dram_tensor("bo", [C], f32, kind="ExternalInput").ap()
    ident = nc.dram_tensor("ident", [128, 128], f32, kind="ExternalInput").ap()
    out = nc.dram_tensor("out", [R, C], f32, kind="ExternalOutput").ap()

    with tile.TileContext(nc) as tc:
        with (
            tc.tile_pool(name="const", bufs=1) as cpool,
            tc.tile_pool(name="resident", bufs=1) as rpool,
            tc.tile_pool(name="xn", bufs=3) as xnpool,
            tc.tile_pool(name="xt", bufs=2) as xtpool,
            tc.tile_pool(name="qy", bufs=2) as qypool,
            tc.tile_pool(name="ob", bufs=2) as obpool,
            tc.tile_pool(name="sc", bufs=2) as scpool,
            tc.tile_pool(name="pt", bufs=2, space="PSUM") as ptpool,
            tc.tile_pool(name="pm", bufs=3, space="PSUM") as pmpool,
            tc.tile_pool(name="po", bufs=3, space="PSUM") as popool,
            tc.tile_pool(name="dram", bufs=1, space="DRAM") as dpool,
        ):
            # ---------- constants ----------
            ident_sb = cpool.tile([128, 128], bf16, tag="ident", name="ident")
            nc.gpsimd.dma_start(ident_sb[:], ident[:])

            def load_w(name, w_ap):
                tiles = []
                for cc in range(NCC):
                    t = cpool.tile([128, HID], bf16, tag=f"{name}_{cc}",
                                   name=f"{name}_{cc}")
                    nc.gpsimd.dma_start(t[:], w_ap[cc * 128:(cc + 1) * 128, :])
                    tiles.append(t)
                return tiles

            Wk_sb = load_w("Wk", Wk)
            Wv_sb = load_w("Wv", Wv)
            Wq_sb = load_w("Wq", Wq)

            # Wo extended with bo as an extra contraction row (ones trick)
            Wo_sb = []
            for kc, ksz in enumerate(HC_SIZES):
                psz = ksz + 1 if kc == 4 else ksz
                t = cpool.tile([psz, C], bf16, tag=f"Wo_{kc}", name=f"Wo_{kc}")
                nc.gpsimd.dma_start(t[0:ksz, :], Wo[kc * 128:kc * 128 + ksz, :])
                if kc == 4:
                    nc.gpsimd.dma_start(t[ksz:ksz + 1, :], bo[None, :])
                Wo_sb.append(t)

            def load_bias(name, b_ap):
                tiles = []
                for hc, hsz in enumerate(HC_SIZES):
                    t = cpool.tile([hsz, 1], f32, tag=f"{name}_{hc}",
                                   name=f"{name}_{hc}")
                    nc.sync.dma_start(t[:], b_ap[hc * 128:hc * 128 + hsz][:, None])
                    tiles.append(t)
                return tiles

            bk_sb = load_bias("bk", bk)
            bv_sb = load_bias("bv", bv)
            bq_sb = load_bias("bq", bq)

            # ---------- resident tensors ----------
            E0 = [rpool.tile([hsz, R], bf16, tag=f"E0_{hc}", name=f"E0_{hc}")
                  for hc, hsz in enumerate(HC_SIZES)]
            Vs = [rpool.tile([hsz, R], bf16, tag=f"V_{hc}", name=f"V_{hc}")
                  for hc, hsz in enumerate(HC_SIZES)]
            Mx = [rpool.tile([hsz, T], bf16, tag=f"M_{hc}", name=f"M_{hc}")
                  for hc, hsz in enumerate(HC_SIZES)]
            Sx = [rpool.tile([hsz, T], bf16, tag=f"S_{hc}", name=f"S_{hc}")
                  for hc, hsz in enumerate(HC_SIZES)]
            den = [rpool.tile([hsz, B_LOC], f32, tag=f"den_{hc}", name=f"den_{hc}")
                   for hc, hsz in enumerate(HC_SIZES)]
            num = [rpool.tile([hsz, B_LOC], f32, tag=f"num_{hc}", name=f"num_{hc}")
                   for hc, hsz in enumerate(HC_SIZES)]
            rr = [rpool.tile([hsz, B_LOC], f32, tag=f"r_{hc}", name=f"r_{hc}")
                  for hc, hsz in enumerate(HC_SIZES)]

            def load_xn(w):
                xn = xnpool.tile([128, NRC * C], bf16, tag="xn", name="xn")
                src = x[w * WIN:(w + 1) * WIN, :].rearrange(
                    "(n p) c -> p n c", p=128)
                nc.gpsimd.dma_start(
                    xn[:].rearrange("p (n c) -> p n c", c=C), src)
                return xn

            def transpose_xt(xn):
                """xt[c_part, cc*WIN + r]; 4 transposes batched per psum tile,
                one DVE copy per cc."""
                xt = xtpool.tile([128, NCC * WIN], bf16, tag="xt", name="xt")
                for cc in range(NCC):
                    pt = ptpool.tile([128, WIN], bf16, tag="pt", name="pt")
                    for rc in range(NRC):
                        nc.tensor.transpose(
                            pt[:, rc * 128:(rc + 1) * 128],
                            xn[:, rc * C + cc * 128: rc * C + (cc + 1) * 128],
                            ident_sb[:])
                    nc.vector.tensor_copy(
                        xt[:, cc * WIN:(cc + 1) * WIN], pt[:])
                return xt

            def project(xt, w_tiles, hc, hsz):
                pm = pmpool.tile([hsz, WIN], f32, tag="pm", name="pm")
                for cc in range(NCC):
                    nc.tensor.matmul(
                        pm[:],
                        w_tiles[cc][:, hc * 128: hc * 128 + hsz],
                        xt[:, cc * WIN:(cc + 1) * WIN],
                        start=(cc == 0), stop=(cc == NCC - 1))
                return pm

            # ---------- pass 1: k (as exp) and v; M accumulated per window ----
            xns = {0: load_xn(0)}
            xts = {0: transpose_xt(xns[0])}
            for w in range(NWIN):
                if w + 1 < NWIN:
                    xns[w + 1] = load_xn(w + 1)
                    xts[w + 1] = transpose_xt(xns[w + 1])
                xt = xts[w]
                for hc, hsz in enumerate(HC_SIZES):
                    pm = project(xt, Wk_sb, hc, hsz)
                    nc.scalar.activation(
                        E0[hc][:, w * WIN:(w + 1) * WIN], pm[:],
                        AF.Exp, bias=bk_sb[hc][:])
                for hc, hsz in enumerate(HC_SIZES):
                    pm = project(xt, Wv_sb, hc, hsz)
                    nc.scalar.activation(
                        Vs[hc][:, w * WIN:(w + 1) * WIN], pm[:],
                        AF.Identity, bias=bv_sb[hc][:])
                # batch-max accumulation (b==0 initializes, else running max)
                for b, lo, hi in _window_segments(w):
                    t0 = w * WIN + lo - b * T
                    t1 = t0 + (hi - lo)
                    for hc, hsz in enumerate(HC_SIZES):
                        e_seg = E0[hc][:, w * WIN + lo: w * WIN + hi]
                        if b == 0:
                            nc.vector.tensor_copy(Mx[hc][:, t0:t1], e_seg)
                        else:
                            nc.vector.tensor_max(
                                Mx[hc][:, t0:t1], Mx[hc][:, t0:t1], e_seg)

            # ---------- prefetch first pass-2 windows (before AR trigger,
            # gpsimd executes in order) ----------
            xn2 = {0: load_xn(0), 1: load_xn(1)}
            xt2 = {0: transpose_xt(xn2[0]), 1: transpose_xt(xn2[1])}

            # ---------- AllReduce(max) over batch dim ----------
            bounce_in = dpool.tile([HID, T], bf16, name="bounce_in")
            bounce_out = dpool.tile([HID, T], bf16, name="bounce_out",
                                    addr_space="Shared")
            for hc, hsz in enumerate(HC_SIZES):
                nc.sync.dma_start(
                    bounce_in[hc * 128:hc * 128 + hsz, :], Mx[hc][:])
            nc.gpsimd.collective_compute(
                "AllReduce",
                mybir.AluOpType.max,
                replica_groups=[list(range(N_CORES))],
                ins=[bounce_in.opt()],
                outs=[bounce_out.opt()],
            )
            for hc, hsz in enumerate(HC_SIZES):
                nc.sync.dma_start(
                    Mx[hc][:], bounce_out[hc * 128:hc * 128 + hsz, :])
                srec = scpool.tile([hsz, T], f32, tag="srec", name="srec")
                nc.vector.reciprocal(srec[:], Mx[hc][:])
                nc.vector.tensor_copy(Sx[hc][:], srec[:])

            def nd_batch(b):
                """num/den/r for one batch across all h chunks."""
                for hc, hsz in enumerate(HC_SIZES):
                    e_b = E0[hc][:, b * T:(b + 1) * T]
                    v_b = Vs[hc][:, b * T:(b + 1) * T]
                    d_b = den[hc][:, b:b + 1]
                    n_b = num[hc][:, b:b + 1]
                    r_b = rr[hc][:, b:b + 1]
                    nc.vector.tensor_mul(e_b, e_b, Sx[hc][:])
                    nc.vector.reduce_sum(d_b, e_b, axis=mybir.AxisListType.X)
                    nc.vector.tensor_mul(e_b, e_b, v_b)
                    nc.vector.reduce_sum(n_b, e_b, axis=mybir.AxisListType.X)
                    nc.vector.reciprocal(r_b, d_b)
                    nc.vector.tensor_mul(r_b, r_b, n_b)

            # ---------- pass 2: q -> sigmoid -> y -> out, nd interleaved ----
            for w in range(NWIN):
                if w + 2 < NWIN:
                    xn2[w + 2] = load_xn(w + 2)
                    xt2[w + 2] = transpose_xt(xn2[w + 2])
                for b in range(B_LOC):
                    if (b * T) // WIN == w:
                        nd_batch(b)
                xt = xt2[w]
                qy = []
                for hc, hsz in enumerate(HC_SIZES):
                    psz = hsz + 1 if hc == 4 else hsz
                    t = qypool.tile([psz, WIN], bf16, tag=f"qy_{hc}",
                                    name=f"qy_{hc}")
                    pm = project(xt, Wq_sb, hc, hsz)
                    nc.scalar.activation(
                        t[0:hsz, :], pm[:], AF.Sigmoid, bias=bq_sb[hc][:])
                    if hc == 4:
                        nc.vector.memset(t[hsz:hsz + 1, :], 1.0)
                    qy.append(t)
                # y = sigmoid(q) * r  (r constant over t within one batch)
                for b, lo, hi in _window_segments(w):
                    for hc, hsz in enumerate(HC_SIZES):
                        nc.vector.tensor_scalar_mul(
                            qy[hc][0:hsz, lo:hi],
                            qy[hc][0:hsz, lo:hi],
                            rr[hc][:, b:b + 1])
                # out = y_ext @ Wo_ext
                for rc in range(NRC):
                    ob = obpool.tile([128, C], f32, tag="ob", name="ob")
                    for nn in range(2):
                        po = popool.tile([128, NOUT_HALF], f32,
                                         tag="po", name="po")
                        for kc, ksz in enumerate(HC_SIZES):
                            psz = ksz + 1 if kc == 4 else ksz
                            nc.tensor.matmul(
                                po[:],
                                qy[kc][0:psz, rc * 128:(rc + 1) * 128],
                                Wo_sb[kc][0:psz,
                                          nn * NOUT_HALF:(nn + 1) * NOUT_HALF],
                                start=(kc == 0), stop=(kc == 4))
                        nc.scalar.copy(
                            ob[:, nn * NOUT_HALF:(nn + 1) * NOUT_HALF], po[:])
                    nc.sync.dma_start(
                        out[w * WIN + rc * 128: w * WIN + (rc + 1) * 128, :],
                        ob[:])

    nc.compile()
    return nc


def kernel(**inputs):
    global LAST_EXEC_NS
    from concourse import bass_utils

    if "nc" not in _CACHE:
        _CACHE["nc"] = _build()
    nc = _CACHE["nc"]

    x = np.asarray(inputs["x"], dtype=np.float32).reshape(B, T, C)
    eye = np.eye(128, dtype=np.float32)
    common = {
        "Wq": np.asarray(inputs["Wq"], np.float32),
        "Wk": np.asarray(inputs["Wk"], np.float32),
        "Wv": np.asarray(inputs["Wv"], np.float32),
        "bq": np.asarray(inputs["bq"], np.float32),
        "bk": np.asarray(inputs["bk"], np.float32),
        "bv": np.asarray(inputs["bv"], np.float32),
        "Wo": np.asarray(inputs["Wo"], np.float32),
        "bo": np.asarray(inputs["bo"], np.float32),
        "ident": eye,
    }
    in_maps = []
    for i in range(N_CORES):
        m = dict(common)
        m["x"] = np.ascontiguousarray(
            x[i * B_LOC:(i + 1) * B_LOC].reshape(R, C))
        in_maps.append(m)

    trace = bool(os.environ.get("KERNEL_TRACE"))
    res = bass_utils.run_bass_kernel_spmd(
        nc, in_maps, core_ids=list(range(N_CORES)), trace=trace)
    LAST_EXEC_NS = res.exec_time_ns

    shards = [res.results[i]["out"].reshape(B_LOC, Hh, Ww, C)
              for i in range(N_CORES)]
    return np.concatenate(shards, axis=0)


# revision 15
# speedup vs baseline: 1.5490x; 1.0285x over previous
"""AFT-Full (nn_AFT_Full) Trainium2 Bass kernel, 8-core SPMD, batch-sharded.

Math note: in the reference, w_bias has shape [1,T,T] and max over dim 0 is the
identity, so exp_wb == exp(0) == 1 and the [T,T] matmuls reduce to column sums
over T (u/vp are unused):
    num[b,h] = sum_t exp(k[b,t,h] - m[t,h]) * v[b,t,h]
    den[b,h] = sum_t exp(k[b,t,h] - m[t,h])
    out = (sigmoid(q) * num/den) @ Wo + bo
where m = max over the FULL batch of k -> cross-core AllReduce(max).

We compute E0 = exp(k + bk) directly (fused into the PSUM->SBUF copy), take
M = max_b E0 (exp is monotone, so this is exp(m)), AllReduce(max) on M, and use
s = 1/M so that exp(k - m) == E0 * s.

Schedule: two passes over x (k,v then q) with on-chip TE transposes; the
batch-max M is accumulated per window during pass 1 so the collective fires
immediately after; pass-2 x loads are issued before the collective trigger
(gpsimd is in-order) and num/den chunks interleave with pass-2 windows.
"""
import os
import sys

sys.path.insert(0, "/opt/trn_rl_repo")

import numpy as np

# ---- problem constants (hardcoded per spec) ----
B, Hh, Ww, C = 64, 24, 24, 768
HID = 576
T = Hh * Ww          # 576
N_CORES = 8
B_LOC = B // N_CORES  # 8
R = B_LOC * T         # 4608 rows per core
WIN = 512             # row window
NWIN = R // WIN       # 9
NRC = WIN // 128      # 4 row chunks per window
NCC = C // 128        # 6 contraction chunks for projections
HC_SIZES = [128, 128, 128, 128, 64]   # HID = 576 partition chunks
NOUT_HALF = 384       # out matmul free-dim split (768 = 2*384)

_CACHE = {}
LAST_EXEC_NS = None


def _window_segments(w):
    """Batch segments [(b, lo, hi)] of window w, window-local coords."""
    lo, hi = w * WIN, (w + 1) * WIN
    segs = []
    for b in range(B_LOC):
        s = max(lo, b * T)
        e = min(hi, (b + 1) * T)
        if s < e:
            segs.append((b, s - lo, e - lo))
    return segs


def _build():
    import concourse.bass as bass
    import concourse.mybir as mybir
    from concourse import bacc, tile

    f32 = mybir.dt.float32
    bf16 = mybir.dt.bfloat16
    AF = mybir.ActivationFunctionType

    nc = bacc.Bacc("TRN2", target_bir_lowering=False, debug=False,
                   num_devices=N_CORES)

    x = nc.dram_tensor("x", [R, C], f32, kind="ExternalInput").ap()
    Wq = nc.dram_tensor("Wq", [C, HID], f32, kind="ExternalInput").ap()
    Wk = nc.dram_tensor("Wk", [C, HID], f32, kind="ExternalInput").ap()
    Wv = nc.dram_tensor("Wv", [C, HID], f32, kind="ExternalInput").ap()
    bq = nc.dram_tensor("bq", [HID], f32, kind="ExternalInput").ap()
    bk = nc.dram_tensor("bk", [HID], f32, kind="ExternalInput").ap()
    bv = nc.dram_tensor("bv", [HID], f32, kind="ExternalInput").ap()
    Wo = nc.dram_tensor("Wo", [HID, C], f32, kind="ExternalInput").ap()
    bo = nc.dram_tensor("bo", [C], f32, kind="ExternalInput").ap()
    ident = nc.dram_tensor("ident", [128, 128], f32, kind="ExternalInput").ap()
    out = nc.dram_tensor("out", [R, C], f32, kind="ExternalOutput").ap()

    with tile.TileContext(nc) as tc:
        with (
            tc.tile_pool(name="const", bufs=1) as cpool,
            tc.tile_pool(name="resident", bufs=1) as rpool,
            tc.tile_pool(name="xn", bufs=4) as xnpool,
            tc.tile_pool(name="xt", bufs=2) as xtpool,
            tc.tile_pool(name="qy", bufs=2) as qypool,
            tc.tile_pool(name="ob", bufs=2) as obpool,
            tc.tile_pool(name="sc", bufs=2) as scpool,
            tc.tile_pool(name="pt", bufs=2, space="PSUM") as ptpool,
            tc.tile_pool(name="pm", bufs=3, space="PSUM") as pmpool,
            tc.tile_pool(name="po", bufs=3, space="PSUM") as popool,
            tc.tile_pool(name="dram", bufs=1, space="DRAM") as dpool,
        ):
            # ---------- constants ----------
            ident_sb = cpool.tile([128, 128], bf16, tag="ident", name="ident")
            nc.gpsimd.dma_start(ident_sb[:], ident[:])

            def load_w(name, w_ap):
                tiles = []
                for cc in range(NCC):
                    t = cpool.tile([128, HID], bf16, tag=f"{name}_{cc}",
                                   name=f"{name}_{cc}")
                    nc.gpsimd.dma_start(t[:], w_ap[cc * 128:(cc + 1) * 128, :])
                    tiles.append(t)
                return tiles

            Wk_sb = load_w("Wk", Wk)
            Wv_sb = load_w("Wv", Wv)
            Wq_sb = load_w("Wq", Wq)

            # Wo extended with bo as an extra contraction row (ones trick)
            Wo_sb = []
            for kc, ksz in enumerate(HC_SIZES):
                psz = ksz + 1 if kc == 4 else ksz
                t = cpool.tile([psz, C], bf16, tag=f"Wo_{kc}", name=f"Wo_{kc}")
                nc.gpsimd.dma_start(t[0:ksz, :], Wo[kc * 128:kc * 128 + ksz, :])
                if kc == 4:
                    nc.gpsimd.dma_start(t[ksz:ksz + 1, :], bo[None, :])
                Wo_sb.append(t)

            def load_bias(name, b_ap):
                tiles = []
                for hc, hsz in enumerate(HC_SIZES):
                    t = cpool.tile([hsz, 1], f32, tag=f"{name}_{hc}",
                                   name=f"{name}_{hc}")
                    nc.sync.dma_start(t[:], b_ap[hc * 128:hc * 128 + hsz][:, None])
                    tiles.append(t)
                return tiles

            bk_sb = load_bias("bk", bk)
            bv_sb = load_bias("bv", bv)
            bq_sb = load_bias("bq", bq)

            # ---------- resident tensors ----------
            E0 = [rpool.tile([hsz, R], bf16, tag=f"E0_{hc}", name=f"E0_{hc}")
                  for hc, hsz in enumerate(HC_SIZES)]
            Vs = [rpool.tile([hsz, R], bf16, tag=f"V_{hc}", name=f"V_{hc}")
                  for hc, hsz in enumerate(HC_SIZES)]
            Mx = [rpool.tile([hsz, T], bf16, tag=f"M_{hc}", name=f"M_{hc}")
                  for hc, hsz in enumerate(HC_SIZES)]
            Sx = [rpool.tile([hsz, T], bf16, tag=f"S_{hc}", name=f"S_{hc}")
                  for hc, hsz in enumerate(HC_SIZES)]
            den = [rpool.tile([hsz, B_LOC], f32, tag=f"den_{hc}", name=f"den_{hc}")
                   for hc, hsz in enumerate(HC_SIZES)]
            num = [rpool.tile([hsz, B_LOC], f32, tag=f"num_{hc}", name=f"num_{hc}")
                   for hc, hsz in enumerate(HC_SIZES)]
            rr = [rpool.tile([hsz, B_LOC], f32, tag=f"r_{hc}", name=f"r_{hc}")
                  for hc, hsz in enumerate(HC_SIZES)]

            def load_xn(w):
                xn = xnpool.tile([128, NRC * C], bf16, tag="xn", name="xn")
                src = x[w * WIN:(w + 1) * WIN, :].rearrange(
                    "(n p) c -> p n c", p=128)
                nc.gpsimd.dma_start(
                    xn[:].rearrange("p (n c) -> p n c", c=C), src)
                return xn

            def transpose_xt(xn):
                """xt[c_part, cc*WIN + r]; 4 transposes batched per psum tile,
                one DVE copy per cc."""
                xt = xtpool.tile([128, NCC * WIN], bf16, tag="xt", name="xt")
                for cc in range(NCC):
                    pt = ptpool.tile([128, WIN], bf16, tag="pt", name="pt")
                    for rc in range(NRC):
                        nc.tensor.transpose(
                            pt[:, rc * 128:(rc + 1) * 128],
                            xn[:, rc * C + cc * 128: rc * C + (cc + 1) * 128],
                            ident_sb[:])
                    nc.vector.tensor_copy(
                        xt[:, cc * WIN:(cc + 1) * WIN], pt[:])
                return xt

            def project(xt, w_tiles, hc, hsz):
                pm = pmpool.tile([hsz, WIN], f32, tag="pm", name="pm")
                for cc in range(NCC):
                    nc.tensor.matmul(
                        pm[:],
                        w_tiles[cc][:, hc * 128: hc * 128 + hsz],
                        xt[:, cc * WIN:(cc + 1) * WIN],
                        start=(cc == 0), stop=(cc == NCC - 1))
                return pm

            # ---------- pass 1: k (as exp) and v; M accumulated per window ----
            xns = {0: load_xn(0)}
            xts = {0: transpose_xt(xns[0])}
            for w in range(NWIN):
                if w + 1 < NWIN:
                    xns[w + 1] = load_xn(w + 1)
                    xts[w + 1] = transpose_xt(xns[w + 1])
                xt = xts[w]
                for hc, hsz in enumerate(HC_SIZES):
                    pm = project(xt, Wk_sb, hc, hsz)
                    nc.scalar.activation(
                        E0[hc][:, w * WIN:(w + 1) * WIN], pm[:],
                        AF.Exp, bias=bk_sb[hc][:])
                for hc, hsz in enumerate(HC_SIZES):
                    pm = project(xt, Wv_sb, hc, hsz)
                    nc.scalar.activation(
                        Vs[hc][:, w * WIN:(w + 1) * WIN], pm[:],
                        AF.Identity, bias=bv_sb[hc][:])
                # batch-max accumulation (b==0 initializes, else running max)
                for b, lo, hi in _window_segments(w):
                    t0 = w * WIN + lo - b * T
                    t1 = t0 + (hi - lo)
                    for hc, hsz in enumerate(HC_SIZES):
                        e_seg = E0[hc][:, w * WIN + lo: w * WIN + hi]
                        if b == 0:
                            nc.vector.tensor_copy(Mx[hc][:, t0:t1], e_seg)
                        else:
                            nc.vector.tensor_max(
                                Mx[hc][:, t0:t1], Mx[hc][:, t0:t1], e_seg)

            # ---------- prefetch first pass-2 windows (before AR trigger,
            # gpsimd executes in order) ----------
            xn2 = {0: load_xn(0), 1: load_xn(1), 2: load_xn(2)}
            xt2 = {0: transpose_xt(xn2[0]), 1: transpose_xt(xn2[1])}

            # ---------- AllReduce(max) over batch dim ----------
            bounce_in = dpool.tile([HID, T], bf16, name="bounce_in")
            bounce_out = dpool.tile([HID, T], bf16, name="bounce_out",
                                    addr_space="Shared")
            for hc, hsz in enumerate(HC_SIZES):
                nc.sync.dma_start(
                    bounce_in[hc * 128:hc * 128 + hsz, :], Mx[hc][:])
            nc.gpsimd.collective_compute(
                "AllReduce",
                mybir.AluOpType.max,
                replica_groups=[list(range(N_CORES))],
                ins=[bounce_in.opt()],
                outs=[bounce_out.opt()],
            )
            for hc, hsz in enumerate(HC_SIZES):
                nc.sync.dma_start(
                    Mx[hc][:], bounce_out[hc * 128:hc * 128 + hsz, :])
                # s = 1/M computed as exp(-ln(M)) on the scalar engine
                lnm = scpool.tile([hsz, T], f32, tag="lnm", name="lnm")
                nc.scalar.activation(lnm[:], Mx[hc][:], AF.Ln)
                nc.scalar.activation(Sx[hc][:], lnm[:], AF.Exp, scale=-1.0)

            def nd_batch(b):
                """num/den/r for one batch across all h chunks."""
                for hc, hsz in enumerate(HC_SIZES):
                    e_b = E0[hc][:, b * T:(b + 1) * T]
                    v_b = Vs[hc][:, b * T:(b + 1) * T]
                    d_b = den[hc][:, b:b + 1]
                    n_b = num[hc][:, b:b + 1]
                    r_b = rr[hc][:, b:b + 1]
                    nc.vector.tensor_mul(e_b, e_b, Sx[hc][:])
                    nc.vector.reduce_sum(d_b, e_b, axis=mybir.AxisListType.X)
                    nc.vector.tensor_mul(e_b, e_b, v_b)
                    nc.vector.reduce_sum(n_b, e_b, axis=mybir.AxisListType.X)
                    nc.vector.reciprocal(r_b, d_b)
                    nc.vector.tensor_mul(r_b, r_b, n_b)

            # ---------- pass 2: q -> sigmoid -> y -> out, nd interleaved ----
            for w in range(NWIN):
                if w + 3 < NWIN:
                    xn2[w + 3] = load_xn(w + 3)
                if w + 2 < NWIN:
                    xt2[w + 2] = transpose_xt(xn2[w + 2])
                for b in range(B_LOC):
                    if (b * T) // WIN == w:
                        nd_batch(b)
                xt = xt2[w]
                qy = []
                for hc, hsz in enumerate(HC_SIZES):
                    psz = hsz + 1 if hc == 4 else hsz
                    t = qypool.tile([psz, WIN], bf16, tag=f"qy_{hc}",
                                    name=f"qy_{hc}")
                    pm = project(xt, Wq_sb, hc, hsz)
                    nc.scalar.activation(
                        t[0:hsz, :], pm[:], AF.Sigmoid, bias=bq_sb[hc][:])
                    if hc == 4:
                        nc.vector.memset(t[hsz:hsz + 1, :], 1.0)
                    qy.append(t)
                # y = sigmoid(q) * r  (r constant over t within one batch)
                for b, lo, hi in _window_segments(w):
                    for hc, hsz in enumerate(HC_SIZES):
                        nc.vector.tensor_scalar_mul(
                            qy[hc][0:hsz, lo:hi],
                            qy[hc][0:hsz, lo:hi],
                            rr[hc][:, b:b + 1])
                # out = y_ext @ Wo_ext
                for rc in range(NRC):
                    ob = obpool.tile([128, C], f32, tag="ob", name="ob")
                    for nn in range(2):
                        po = popool.tile([128, NOUT_HALF], f32,
                                         tag="po", name="po")
                        for kc, ksz in enumerate(HC_SIZES):
                            psz = ksz + 1 if kc == 4 else ksz
                            nc.tensor.matmul(
                                po[:],
                                qy[kc][0:psz, rc * 128:(rc + 1) * 128],
                                Wo_sb[kc][0:psz,
                                          nn * NOUT_HALF:(nn + 1) * NOUT_HALF],
                                start=(kc == 0), stop=(kc == 4))
                        nc.scalar.copy(
                            ob[:, nn * NOUT_HALF:(nn + 1) * NOUT_HALF], po[:])
                    nc.sync.dma_start(
                        out[w * WIN + rc * 128: w * WIN + (rc + 1) * 128, :],
                        ob[:])

    nc.compile()
    return nc


def kernel(**inputs):
    global LAST_EXEC_NS
    from concourse import bass_utils

    if "nc" not in _CACHE:
        _CACHE["nc"] = _build()
    nc = _CACHE["nc"]

    x = np.asarray(inputs["x"], dtype=np.float32).reshape(B, T, C)
    eye = np.eye(128, dtype=np.float32)
    common = {
        "Wq": np.asarray(inputs["Wq"], np.float32),
        "Wk": np.asarray(inputs["Wk"], np.float32),
        "Wv": np.asarray(inputs["Wv"], np.float32),
        "bq": np.asarray(inputs["bq"], np.float32),
        "bk": np.asarray(inputs["bk"], np.float32),
        "bv": np.asarray(inputs["bv"], np.float32),
        "Wo": np.asarray(inputs["Wo"], np.float32),
        "bo": np.asarray(inputs["bo"], np.float32),
        "ident": eye,
    }
    in_maps = []
    for i in range(N_CORES):
        m = dict(common)
        m["x"] = np.ascontiguousarray(
            x[i * B_LOC:(i + 1) * B_LOC].reshape(R, C))
        in_maps.append(m)

    trace = bool(os.environ.get("KERNEL_TRACE"))
    res = bass_utils.run_bass_kernel_spmd(
        nc, in_maps, core_ids=list(range(N_CORES)), trace=trace)
    LAST_EXEC_NS = res.exec_time_ns

    shards = [res.results[i]["out"].reshape(B_LOC, Hh, Ww, C)
              for i in range(N_CORES)]
    return np.concatenate(shards, axis=0)
